# revision 1
# baseline (speedup 1.0000x reference)
"""Trainium2 Bass kernel for nn_CopyGenerator (scatter_memory).

Strategy (8 NeuronCores, tensor-parallel over the vocab dim):
  - Each core owns a 4000-wide vocab slice of logits/prob/output.
  - logits = hidden @ W.T + b computed as a bf16 matmul with an augmented
    contraction dim (K=1025; last row of hiddenT is ones, last row of WT is b).
  - The per-batch scatter-add of copy-attention mass into the vocab dim is
    reformulated as a dense matmul  AE = A_T.T @ E  with host-built operands:
    per vocab tile, E is a one-hot (slot -> vocab col) matrix and A_T holds the
    attention columns masked by batch. Duplicate targets accumulate exactly as
    in the reference (matmul accumulation).
  - softmax denominator = AllReduce(partial exp sums) across the 8 cores
    (one tiny collective per 512-row chunk, overlapped with the next chunk's
    matmuls).
  - final out = ln((exp(l)*s1 + AE*copy + EPS) * (1/Sigma)) with all per-row
    scalars fused into ACT/DVE ops; special columns (copy_idx / pad_idx) are
    handled with host-side W column edits so the SPMD program has no
    per-core control flow.

kernel(**inputs) takes the FULL inputs and returns the FULL (2048, 32000) f32
output; it shards/preps on the host, runs the Bass program on cores 0-7 via
run_bass_kernel_spmd, and concatenates the per-core slices.
"""

import numpy as np
import ml_dtypes

EPS = 1e-10
N_CORES = 8
LB = 2048          # tgt_len * batch rows
D = 1024           # d_model
V = 32000          # vocab
B = 64             # batch
S = 64             # src len
VS = V // N_CORES  # vocab slice per core (4000)
NT = 8             # n-tiles per core
NW = VS // NT      # n-tile width (500)
KS = 128           # scatter slot capacity per (core, n-tile)
KC = 9             # contraction chunks: 8 x 128 + 1 (bias row)
MT = 16            # m-tiles (128 rows each)
CHUNKS = 4         # row chunks (4 m-tiles each) between collectives
BF16 = ml_dtypes.bfloat16

_PROGRAM_CACHE = {}


def _build_program(pad_n, pad_c, single_core=False, compile_=True):
    """Build + compile the SPMD Bass program. pad_n/pad_c: n-tile index and
    column of pad_idx within the owning core's slice (fixup is data-driven via
    the psel input, so every core runs the same code). single_core: build a
    1-device variant (collective replaced by a DMA copy) for TimelineSim."""
    import concourse.tile as tile
    from concourse import bacc, mybir

    f32 = mybir.dt.float32
    bf16 = mybir.dt.bfloat16
    AX = mybir.AxisListType
    OP = mybir.AluOpType
    AF = mybir.ActivationFunctionType

    from concourse.tile_rust import add_dep_helper

    nc = bacc.Bacc("TRN2", target_bir_lowering=False, debug=False,
                   num_devices=1 if single_core else N_CORES)

    ht_ext = nc.dram_tensor("ht", [D + 1, LB], bf16, kind="ExternalInput")
    wtt_ext = nc.dram_tensor("wtt", [8, NT, 128, NW], bf16, kind="ExternalInput")
    wbias_ext = nc.dram_tensor("wbias", [1, VS], bf16, kind="ExternalInput")
    wts_ext = nc.dram_tensor("wts", [D + 1, 2], bf16, kind="ExternalInput")
    at_ext = nc.dram_tensor("at", [NT, KS, LB], f32, kind="ExternalInput")
    e_ext = nc.dram_tensor("e", [NT, KS, NW], f32, kind="ExternalInput")
    sc0_ext = nc.dram_tensor("sc0", [LB, 1], f32, kind="ExternalInput")
    psel_ext = nc.dram_tensor("psel", [128, 1], f32, kind="ExternalInput")
    out_ext = nc.dram_tensor("out", [LB, VS], f32, kind="ExternalOutput")

    with tile.TileContext(nc) as tc:
        with (
            tc.tile_pool(name="const", bufs=1) as const,
            tc.tile_pool(name="wpool", bufs=2) as wpool,
            tc.tile_pool(name="slabp", bufs=8) as slabp,
            tc.tile_pool(name="atp", bufs=6) as atp,
            tc.tile_pool(name="statp", bufs=8) as statp,
            tc.tile_pool(name="bigp", bufs=4) as bigp,
            tc.tile_pool(name="psl", bufs=3, space="PSUM") as psl,
            tc.tile_pool(name="psa", bufs=2, space="PSUM") as psa,
            tc.tile_pool(name="pss", bufs=2, space="PSUM") as pss,
            tc.tile_pool(name="dramp", bufs=2, space="DRAM") as dramp,
        ):
            # ---- residents ----
            ht_sb = []
            wts_sb = []
            for kc in range(KC):
                p = 128 if kc < 8 else 1
                h = const.tile([p, LB], bf16, tag=f"ht{kc}", name=f"ht{kc}")
                nc.sync.dma_start(h[:], ht_ext[kc * 128:kc * 128 + p, :])
                ht_sb.append(h)
                w = const.tile([p, 2], bf16, tag=f"wts{kc}", name=f"wts{kc}")
                nc.sync.dma_start(w[:], wts_ext[kc * 128:kc * 128 + p, :])
                wts_sb.append(w)
            e_sb = []
            for t in range(NT):
                e = const.tile([KS, NW], f32, tag=f"e{t}", name=f"e{t}")
                nc.sync.dma_start(e[:], e_ext[t])
                e_sb.append(e)
            psel_sb = const.tile([128, 1], f32, tag="psel", name="psel_sb")
            nc.sync.dma_start(psel_sb[:], psel_ext.ap())
            bt = const.tile([1, 1], f32, tag="bt", name="bt")

            # ACT-table thrash control: all Exp of chunk c wait on a tiny Exp
            # barrier that follows every Ln of chunk c-1, so the scalar engine
            # runs [Exp block][Ln block] per chunk (2 table loads per chunk)
            # instead of alternating Exp/Ln (one ~1.4us table load each).
            act_barrier = None

            for c in range(CHUNKS):
                ms = [4 * c + i for i in range(4)]
                # ---- phase 1: specials, logits matmul, exp -> slab ----
                slab = {}
                pstat = {}
                lcp = {}
                for m in ms:
                    slab[m] = slabp.tile([128, VS], bf16, tag="slab",
                                         name=f"slab{m}")
                    pstat[m] = statp.tile([128, NT], f32, tag="pstat",
                                          name=f"pstat{m}")
                    ps_s = pss.tile([128, 2], f32, tag="pss", name=f"pss{m}")
                    for kc in range(KC):
                        nc.tensor.matmul(
                            ps_s[:], ht_sb[kc][:, m * 128:(m + 1) * 128],
                            wts_sb[kc][:], start=(kc == 0), stop=(kc == KC - 1))
                    t_lcp = statp.tile([128, 2], f32, tag="lcp", name=f"lcp{m}")
                    nc.vector.tensor_copy(t_lcp[:], ps_s[:])
                    lcp[m] = t_lcp
                for n in range(NT):
                    wt = []
                    for kc in range(KC):
                        p = 128 if kc < 8 else 1
                        w = wpool.tile([p, NW], bf16, tag=f"w{kc}",
                                       name=f"w{kc}_{c}_{n}")
                        if kc < 8:
                            nc.sync.dma_start(w[:], wtt_ext[kc, n])
                        else:
                            nc.sync.dma_start(
                                w[:], wbias_ext[:, n * NW:(n + 1) * NW])
                        wt.append(w)
                    for m in ms:
                        pl = psl.tile([128, NW], f32, tag="psl",
                                      name=f"psl{m}_{n}")
                        for kc in range(KC):
                            nc.tensor.matmul(
                                pl[:], ht_sb[kc][:, m * 128:(m + 1) * 128],
                                wt[kc][:], start=(kc == 0), stop=(kc == KC - 1))
                        ei = nc.scalar.activation(
                            slab[m][:, n * NW:(n + 1) * NW], pl[:], AF.Exp,
                            accum_out=pstat[m][:, n:n + 1])
                        if act_barrier is not None:
                            add_dep_helper(ei.ins, act_barrier.ins,
                                           reason="act exp after ln block")
                # ---- phase 2: per-row partial sums -> AllReduce ----
                cin = dramp.tile([128, 4], f32, tag="cin", name=f"cin{c}")
                cout = dramp.tile([128, 4], f32, tag="cout", name=f"cout{c}")
                pall = statp.tile([128, 4], f32, tag="pall", name=f"pall{c}")
                explp = {}
                for i, m in enumerate(ms):
                    par = statp.tile([128, 1], f32, tag="par", name=f"par{m}")
                    nc.vector.tensor_reduce(par[:], pstat[m][:], axis=AX.X,
                                            op=OP.add)
                    elp = statp.tile([128, 1], f32, tag="explp",
                                     name=f"explp{m}")
                    ei = nc.scalar.activation(elp[:], lcp[m][:, 1:2], AF.Exp)
                    if act_barrier is not None:
                        add_dep_helper(ei.ins, act_barrier.ins,
                                       reason="act exp after ln block")
                    explp[m] = elp
                    # partial += psel * (exp(lp) - 1): core owning pad_idx had
                    # exp(0)=1 at that col; the true exp(l_pad) must be in the
                    # denominator instead.
                    t1 = statp.tile([128, 1], f32, tag="t1", name=f"t1_{m}")
                    nc.vector.tensor_scalar_add(t1[:], elp[:], -1.0)
                    t2 = statp.tile([128, 1], f32, tag="t2", name=f"t2_{m}")
                    nc.vector.tensor_mul(t2[:], t1[:], psel_sb[:])
                    nc.vector.tensor_add(pall[:, i:i + 1], par[:], t2[:])
                nc.sync.dma_start(cin[:], pall[:])
                if single_core:
                    nc.sync.dma_start(cout[:], cin[:])
                else:
                    nc.gpsimd.collective_compute(
                        "AllReduce", OP.add,
                        replica_groups=[list(range(N_CORES))],
                        ins=[cin.opt()], outs=[cout.opt()])
                den = statp.tile([128, 4], f32, tag="den", name=f"den{c}")
                nc.sync.dma_start(den[:], cout[:])
                # ---- phase 3: stats, AE matmul, fuse, ln, store ----
                ln_insts = []
                for i, m in enumerate(ms):
                    en = statp.tile([128, 1], f32, tag="en", name=f"en{m}")
                    ei = nc.scalar.activation(en[:], lcp[m][:, 0:1], AF.Exp,
                                              scale=-1.0)     # exp(-lc)
                    if act_barrier is not None:
                        add_dep_helper(ei.ins, act_barrier.ins,
                                       reason="act exp after ln block")
                    en1 = statp.tile([128, 1], f32, tag="en1", name=f"en1{m}")
                    nc.vector.tensor_scalar_add(en1[:], en[:], 1.0)
                    cpy = statp.tile([128, 1], f32, tag="cpy", name=f"cpy{m}")
                    nc.vector.reciprocal(cpy[:], en1[:])      # sigmoid(lc)
                    omc = statp.tile([128, 1], f32, tag="omc", name=f"omc{m}")
                    nc.vector.tensor_scalar(omc[:], cpy[:], -1.0, 1.0,
                                            op0=OP.mult, op1=OP.add)
                    invd = statp.tile([128, 1], f32, tag="invd",
                                      name=f"invd{m}")
                    nc.vector.reciprocal(invd[:], den[:, i:i + 1])
                    s1 = statp.tile([128, 1], f32, tag="s1", name=f"s1_{m}")
                    nc.vector.tensor_mul(s1[:], omc[:], invd[:])
                    sc0m = statp.tile([128, 1], f32, tag="sc0m",
                                      name=f"sc0m{m}")
                    nc.sync.dma_start(sc0m[:],
                                      sc0_ext[m * 128:(m + 1) * 128, :])
                    u1 = statp.tile([128, 1], f32, tag="u1", name=f"u1_{m}")
                    nc.vector.tensor_mul(u1[:], s1[:], explp[m][:])
                    u2 = statp.tile([128, 1], f32, tag="u2", name=f"u2_{m}")
                    nc.vector.tensor_mul(u2[:], cpy[:], sc0m[:])
                    u3 = statp.tile([128, 1], f32, tag="u3", name=f"u3_{m}")
                    nc.vector.tensor_add(u3[:], u1[:], u2[:])
                    sg = statp.tile([128, 1], f32, tag="sg", name=f"sg{m}")
                    nc.vector.tensor_scalar(sg[:], u3[:], -1.0, 1.0 + EPS,
                                            op0=OP.mult, op1=OP.add)
                    invs = statp.tile([128, 1], f32, tag="invs",
                                      name=f"invs{m}")
                    nc.vector.reciprocal(invs[:], sg[:])
                    fix = statp.tile([128, 1], f32, tag="fix", name=f"fix{m}")
                    nc.vector.tensor_scalar(fix[:], sg[:], EPS, EPS,
                                            op0=OP.mult, op1=OP.add)
                    for n in range(NT):
                        at = atp.tile([KS, 128], f32, tag="at",
                                      name=f"at{m}_{n}")
                        nc.sync.dma_start(at[:],
                                          at_ext[n][:, m * 128:(m + 1) * 128])
                        pa = psa.tile([128, NW], f32, tag="psa",
                                      name=f"psa{m}_{n}")
                        nc.tensor.matmul(pa[:], at[:], e_sb[n][:],
                                         start=True, stop=True)
                        sb2 = bigp.tile([128, NW], f32, tag="sb2",
                                        name=f"sb2_{m}_{n}")
                        nc.vector.tensor_scalar(sb2[:], pa[:], cpy[:], EPS,
                                                op0=OP.mult, op1=OP.add)
                        sb3 = bigp.tile([128, NW], f32, tag="sb3",
                                        name=f"sb3_{m}_{n}")
                        nc.vector.scalar_tensor_tensor(
                            sb3[:], slab[m][:, n * NW:(n + 1) * NW], s1[:],
                            sb2[:], op0=OP.mult, op1=OP.add)
                        if n == pad_n:
                            # data-driven pad-col fixup (psel=0 -> no-op):
                            # sb3[:,pc] += psel * (fix - sb3[:,pc])
                            d1 = statp.tile([128, 1], f32, tag="d1",
                                            name=f"d1_{m}")
                            nc.vector.tensor_sub(
                                d1[:], fix[:], sb3[:, pad_c:pad_c + 1])
                            d2 = statp.tile([128, 1], f32, tag="d2",
                                            name=f"d2_{m}")
                            nc.vector.tensor_mul(d2[:], d1[:], psel_sb[:])
                            nc.vector.tensor_add(
                                sb3[:, pad_c:pad_c + 1],
                                sb3[:, pad_c:pad_c + 1], d2[:])
                        osb = bigp.tile([128, NW], f32, tag="osb",
                                        name=f"osb{m}_{n}")
                        li = nc.scalar.activation(osb[:], sb3[:], AF.Ln,
                                                  scale=invs[:])
                        ln_insts.append(li)
                        nc.sync.dma_start(
                            out_ext[m * 128:(m + 1) * 128,
                                    n * NW:(n + 1) * NW], osb[:])
                if c < CHUNKS - 1:
                    # Exp barrier after this chunk's Ln block (pre-warms the
                    # Exp table for the next chunk's block).
                    act_barrier = nc.scalar.activation(bt[:], psel_sb[0:1, :],
                                                       AF.Exp)
                    for li in ln_insts:
                        add_dep_helper(act_barrier.ins, li.ins,
                                       reason="barrier after ln block")

    if compile_:
        nc.compile()
    return nc


def _host_prep(hidden, attn, W, b, src, alignment, copy_idx, pad_idx):
    hidden = np.asarray(hidden, np.float32)
    attn = np.asarray(attn, np.float32)
    W = np.asarray(W, np.float32)
    b = np.asarray(b, np.float32)
    src = np.asarray(src)
    alignment = np.asarray(alignment)
    copy_idx = int(copy_idx)
    pad_idx = int(pad_idx)

    tgt = alignment[src[:, :, 0]].T.astype(np.int64)   # (B, S)

    sc0 = np.zeros((LB, 1), np.float32)
    pad_mask = tgt == pad_idx                          # (B, S)
    for bb in range(B):
        if pad_mask[bb].any():
            sc0[bb::B, 0] = attn[bb::B][:, pad_mask[bb]].sum(axis=1)

    hT_aug = np.empty((D + 1, LB), np.float32)
    hT_aug[:D] = hidden.T
    hT_aug[D] = 1.0
    hT_bf = np.ascontiguousarray(hT_aug.astype(BF16))

    in_maps = []
    for k in range(N_CORES):
        lo = k * VS
        WT = np.empty((D, VS + 2), np.float32)
        WT[:, :VS] = W[lo:lo + VS].T
        WT[:, VS] = W[copy_idx]
        WT[:, VS + 1] = W[pad_idx]
        bias = np.empty(VS + 2, np.float32)
        bias[:VS] = b[lo:lo + VS]
        bias[VS] = b[copy_idx]
        bias[VS + 1] = b[pad_idx]
        if lo <= copy_idx < lo + VS:
            WT[:, copy_idx - lo] = 0.0
            bias[copy_idx - lo] = EPS
        if lo <= pad_idx < lo + VS:
            WT[:, pad_idx - lo] = 0.0
            bias[pad_idx - lo] = 0.0
        WT_aug = np.concatenate([WT, bias[None, :]], axis=0).astype(BF16)

        wtt = np.empty((8, NT, 128, NW), BF16)
        for kc in range(8):
            for n in range(NT):
                wtt[kc, n] = WT_aug[kc * 128:(kc + 1) * 128,
                                    n * NW:(n + 1) * NW]
        wbias = np.ascontiguousarray(WT_aug[D:D + 1, :VS])
        wts = np.ascontiguousarray(WT_aug[:, VS:VS + 2])

        AT = np.zeros((NT, KS, LB), np.float32)
        E = np.zeros((NT, KS, NW), np.float32)
        counts = np.zeros(NT, np.int64)
        in_slice = (tgt >= lo) & (tgt < lo + VS) & (tgt != pad_idx)
        bs, ss = np.nonzero(in_slice)
        for bb, s in zip(bs, ss):
            tv = tgt[bb, s]
            t = (tv - lo) // NW
            j = counts[t]
            assert j < KS, f"scatter slot overflow: core {k} tile {t}"
            counts[t] = j + 1
            AT[t, j, bb::B] = attn[bb::B, s]
            E[t, j, (tv - lo) % NW] = 1.0

        psel = np.full((128, 1),
                       1.0 if lo <= pad_idx < lo + VS else 0.0, np.float32)
        in_maps.append({
            "ht": hT_bf,
            "wtt": wtt,
            "wbias": wbias,
            "wts": wts,
            "at": AT,
            "e": E,
            "sc0": sc0,
            "psel": psel,
        })
    pad_core = pad_idx // VS
    pad_n = (pad_idx % VS) // NW
    pad_c = (pad_idx % VS) % NW
    return in_maps, pad_n, pad_c


def _run(in_maps, pad_n, pad_c, trace=False):
    from concourse.bass_utils import run_bass_kernel_spmd
    key = (pad_n, pad_c)
    if key not in _PROGRAM_CACHE:
        _PROGRAM_CACHE[key] = _build_program(pad_n, pad_c)
    nc = _PROGRAM_CACHE[key]
    res = run_bass_kernel_spmd(nc, in_maps, list(range(N_CORES)), trace=trace)
    return res


def kernel(hidden, attn, W, b, src, alignment, copy_idx=4, pad_idx=0,
           _trace=False, _return_raw=False):
    in_maps, pad_n, pad_c = _host_prep(hidden, attn, W, b, src, alignment,
                                       copy_idx, pad_idx)
    res = _run(in_maps, pad_n, pad_c, trace=_trace)
    out = np.concatenate([res.results[k]["out"] for k in range(N_CORES)],
                         axis=1)
    if _return_raw:
        return out, res
    return out


# ---------------------------------------------------------------------------
# Benchmarking support (test.py only): time the NEFF with device-resident
# inputs, no donation, min over iters; subtract a null-kernel launch baseline.
# ---------------------------------------------------------------------------

def _make_timed_runner(nc, in_maps, repeat=1):
    import time
    import jax
    from jax.sharding import Mesh, PartitionSpec, NamedSharding
    from jax.experimental.shard_map import shard_map
    from concourse import bass2jax, mybir

    bass2jax.install_neuronx_cc_hook()
    partition_name = (nc.partition_id_tensor.name
                      if nc.partition_id_tensor else None)
    in_names, out_names, out_avals, zero_outs = [], [], [], []
    for alloc in nc.m.functions[0].allocations:
        if not isinstance(alloc, mybir.MemoryLocationSet):
            continue
        name = alloc.memorylocations[0].name
        if alloc.kind == "ExternalInput":
            if name != partition_name:
                in_names.append(name)
        elif alloc.kind == "ExternalOutput":
            out_names.append(name)
            shape = tuple(alloc.tensor_shape)
            dtype = mybir.dt.np(alloc.dtype)
            out_avals.append(jax.core.ShapedArray(shape, dtype))
            zero_outs.append(np.zeros(shape, dtype))
    n_params = len(in_names)
    in_names = in_names + out_names
    if partition_name is not None:
        in_names.append(partition_name)

    def _body(*args):
        ins = list(args[:n_params])
        outs = tuple(args[n_params:])
        pid = ([bass2jax.partition_id_tensor()]
               if partition_name is not None else [])

        def one(outs):
            return bass2jax._bass_exec_p.bind(
                *ins, *outs, *pid, out_avals=tuple(out_avals),
                in_names=tuple(in_names), out_names=tuple(out_names),
                lowering_input_output_aliases=(), sim_require_finite=True,
                sim_require_nnan=True, nc=nc)

        if repeat == 1:
            return tuple(one(outs))
        return tuple(jax.lax.fori_loop(0, repeat, lambda i, o: tuple(one(o)),
                                       tuple(outs)))

    n = len(in_maps)
    devices = jax.devices()[:n]
    mesh = Mesh(np.asarray(devices), ("core",))
    spec = PartitionSpec("core")
    sharding = NamedSharding(mesh, spec)
    in_specs = (spec,) * (n_params + len(out_names))
    out_specs = (spec,) * len(out_names)
    fn = jax.jit(shard_map(_body, mesh=mesh, in_specs=in_specs,
                           out_specs=out_specs, check_rep=False),
                 keep_unused=True)
    per_core = [[np.asarray(m[name]) for name in in_names[:n_params]]
                for m in in_maps]
    args = [jax.device_put(
        np.concatenate([per_core[c][i] for c in range(n)], axis=0), sharding)
        for i in range(n_params)]
    args += [jax.device_put(
        np.zeros((n * z.shape[0], *z.shape[1:]), z.dtype), sharding)
        for z in zero_outs]

    def run_once():
        t0 = time.perf_counter()
        outs = fn(*args)
        jax.block_until_ready(outs)
        t1 = time.perf_counter()
        del outs
        return t1 - t0

    return run_once


def _build_null_program():
    """Trivial SPMD NEFF used to estimate launch/RPC overhead."""
    import concourse.tile as tile
    from concourse import bacc, mybir
    f32 = mybir.dt.float32
    nc = bacc.Bacc("TRN2", target_bir_lowering=False, debug=False,
                   num_devices=N_CORES)
    x = nc.dram_tensor("x", [128, 128], f32, kind="ExternalInput")
    y = nc.dram_tensor("y", [128, 128], f32, kind="ExternalOutput")
    with tile.TileContext(nc) as tc:
        with tc.tile_pool(name="p", bufs=1) as p:
            t = p.tile([128, 128], f32)
            nc.sync.dma_start(t[:], x.ap())
            nc.sync.dma_start(y.ap(), t[:])
    nc.compile()
    return nc


def benchmark(hidden, attn, W, b, src, alignment, copy_idx=4, pad_idx=0,
              iters=12, r_lo=1, r_hi=17):
    """Estimate per-execution HW time via the slope between r_lo and r_hi
    chained NEFF executions inside one dispatch (amortizes the ~80ms axon
    RPC overhead). Returns (est_hw_ns, t_lo_list, t_hi_list)."""
    in_maps, pad_n, pad_c = _host_prep(hidden, attn, W, b, src, alignment,
                                       copy_idx, pad_idx)
    key = (pad_n, pad_c)
    if key not in _PROGRAM_CACHE:
        _PROGRAM_CACHE[key] = _build_program(pad_n, pad_c)
    nc = _PROGRAM_CACHE[key]
    run_lo = _make_timed_runner(nc, in_maps, repeat=r_lo)
    run_hi = _make_timed_runner(nc, in_maps, repeat=r_hi)
    t_lo = [run_lo() for _ in range(iters)]
    t_hi = [run_hi() for _ in range(iters)]
    est = (min(t_hi[1:]) - min(t_lo[1:])) / (r_hi - r_lo)
    return int(est * 1e9), t_lo, t_hi

